# revision 48
# baseline (speedup 1.0000x reference)
"""Multi-head attention (16 heads, D=1024, B=2, S=2048) on 8 TRN2 NeuronCores.

Sharding: tensor-parallel over heads. Each core owns 2 heads (128 features):
W_q/k/v column-sliced, W_o row-sliced; fp16 partial outputs summed on host.

Per-core dataflow, everything "transposed" (features on partitions) so the
key-padding mask folds into the ACT exp bias and the attention matrix lands
directly in the layout the A@V matmul needs:

  QT/KT[f,s] = W^T @ x^T           (PE, contraction d on partitions)
  VT -> PE-transpose -> V[s,f]     (natural, k on partitions, + ones cols)
  scores^T[k,q] = KT_h^T . QT_h    (per head, k-chunks of 128)
  attn^T = exp(scores*0.125 + mask_bias[k])    (ACT, PSUM->SBUF, fp16)
  av_h[f,q] += [V_h|1]^T . attn_h^T   (PSUM accum over k-chunks; rowsum
                                       rides free at partition 64)
  normalize: av copied to SBUF (ACT) freeing the PSUM banks, 1/rowsum on
  DVE (fp16), K=1 matmul broadcasts 1/r over 64 partitions, DVE applies
  it SBUF x PSUM. head1's normalized tile is shifted to partitions 64:128
  with a K=64 identity matmul (PE is the only partition mover; matmul
  PSUM writes must start at partition 0/32/64), giving ONE [128, q] outT
  tile so W_o runs as single K=128 matmuls -- half the row-streams of a
  split-head contraction (PE matmul cost is output-columns only; the
  contraction depth K does not enter the cost).

Scheduling (cost-model engine rates: PE 0.4167ns/row, ACT 0.833ns/elem,
DVE 1.04ns/elem, DMA 360GB/s shared + 625ns HWDGE issue per transfer):
  - x arrives in 4-kc-chunk batched DMAs (smaller DMAs are HWDGE-bound)
  - weight wall sections (wk|wq|wv|wo packed p-major, 2KB elements) are
    DMA'd lazily right before their first consumer
  - per batch, K/V tiles stream in 512-token segments with qt0's attention
    chunks interleaved, so PE works on early k-chunks while later x tiles
    are still in flight; K/V token ranges are trimmed to valid_len
  - each q tile's normalize chain + W_o matmuls + PSUM drains are deferred
    and popped one-per-k-chunk inside the NEXT q tile's attention, hiding
    the serial recip/broadcast chain and the PSUM drains under exp-paced
    slack; Q projections are emitted two q tiles ahead so their PSUM
    drain never gates the first scores of a tile
  - b1's first K projection is hoisted into b0's last q-tile boundary

Key-padding mask: k-chunks entirely beyond valid_len are skipped (program
is specialized to the valid_lens values at call time); the boundary chunk
uses a -1e6 additive bias inside the exp activation (exp underflows to 0).
"""

import math
import os

import ml_dtypes
import numpy as np

B = 2
S = 2048
D = 1024
NT = B * S          # 4096 rows, b-major
F = 128             # features per core (2 heads x 64)
DH = 64
P = 128
DK = D // P         # 8 contraction chunks for projections
N_CORES = 8
NEG = -1e6

# byte offsets (in elements) of each weight block inside the packed wall
WQ_BASE = 0
WK_BASE = DK * F
WV_BASE = 2 * DK * F
WO_BASE = 3 * DK * F
WALL_W = 3 * DK * F + D

_CACHE: dict = {}
MM_LABELS: list = []


def _build_program(KC: tuple[int, int], cfg: dict):
    import concourse.bass as bass
    import concourse.tile as tile
    from concourse import mybir
    from concourse.masks import make_identity

    dt = mybir.dt
    DT_IN = getattr(dt, cfg["dt_in"])        # xT + W in HBM / matmul dtype
    DT_ATTN = getattr(dt, cfg["dt_attn"])    # attn / V / QT / KT storage
    DT_OUT = getattr(dt, cfg["dt_out"])      # partial output in HBM

    nc = bass.Bass("TRN2")
    MM_LABELS.clear()
    _real_mm = nc.tensor.matmul
    _real_tp = nc.tensor.transpose

    def _mm(*a, _lab=None, **k):
        MM_LABELS.append(_mm_label[0])
        return _real_mm(*a, **k)

    def _tp(*a, **k):
        return _real_tp(*a, **k)

    _mm_label = ["?"]
    nc.tensor.matmul = _mm
    nc.tensor.transpose = _tp

    def _lab(s):
        _mm_label[0] = s

    xtq_d = nc.dram_tensor("xtq", [D, NT], DT_IN, kind="ExternalInput")
    xtk_d = nc.dram_tensor("xtk", [D, NT], DT_IN, kind="ExternalInput")
    xtv_d = nc.dram_tensor("xtv", [D, NT], DT_IN, kind="ExternalInput")
    wall_d = nc.dram_tensor("wall", [P, WALL_W], DT_IN, kind="ExternalInput")
    mask_d = nc.dram_tensor("maskt", [P, B * 16], dt.float32, kind="ExternalInput")
    out_d = nc.dram_tensor("out_part", [NT, D], DT_OUT, kind="ExternalOutput")

    from contextlib import ExitStack

    B_X = int(cfg.get("b_x", 3))        # x streaming tiles per kc tag
    B_AT = int(cfg.get("b_at", 6))      # attn tiles
    B_SC = int(cfg.get("b_sc", 2))      # score psum bufs (2 banks each)
    B_AV = 2                            # av psum banks (both live per qt)
    B_PW = int(cfg.get("b_pw", 2))      # shared proj/bc/wo psum banks
    assert 2 * B_SC + B_AV + B_PW <= 8

    with tile.TileContext(nc) as tc, ExitStack() as ctx:
        const = ctx.enter_context(tc.tile_pool(name="const", bufs=1))
        xpool = ctx.enter_context(tc.tile_pool(name="xpool", bufs=B_X))
        apool = ctx.enter_context(tc.tile_pool(name="apool", bufs=B_AT))
        rpool = ctx.enter_context(tc.tile_pool(name="rpool", bufs=2))
        opool = ctx.enter_context(tc.tile_pool(name="opool", bufs=4))
        ps_score = ctx.enter_context(
            tc.tile_pool(name="ps_score", bufs=B_SC, space="PSUM"))
        ps_av = ctx.enter_context(
            tc.tile_pool(name="ps_av", bufs=B_AV, space="PSUM"))
        ps_pw = ctx.enter_context(
            tc.tile_pool(name="ps_pw", bufs=B_PW, space="PSUM"))

        # ---- constants ----
        # Matmult instructions tolerate only ONE sync-wait, so tensors a
        # matmul reads are written by DVE (one mergeable semaphore): the
        # packed weight wall bounces DRAM -> raw tile -> DVE copy -> tile.
        wall_raw = const.tile([P, WALL_W], DT_IN, tag="wall_raw")
        wall = const.tile([P, WALL_W], DT_IN, tag="wall")
        _wsec_done = set()

        def w_sec(base, width=DK * F):
            # DMA+copy one weight section, emitted lazily right before its
            # first consumer so the DMA queue stays in true dependency order
            if base in _wsec_done:
                return
            _wsec_done.add(base)
            nc.sync.dma_start(wall_raw[:, base:base + width],
                              wall_d[:, base:base + width])
            nc.vector.tensor_copy(out=wall[:, base:base + width],
                                  in_=wall_raw[:, base:base + width])

        mask_raw = const.tile([P, B * 16], dt.float32, tag="mask_raw")
        mask_sb = const.tile([P, B * 16], dt.float32, tag="mask")

        ident_g = const.tile([P, P], DT_ATTN, tag="ident_g")
        make_identity(nc, ident_g)
        ident = const.tile([P, P], DT_ATTN, tag="ident")
        nc.vector.tensor_copy(out=ident, in_=ident_g)

        # ones row at p64 (where both rowsums live) for the K=1 1/r
        # broadcast matmuls
        onesk = const.tile([P, DH], DT_ATTN, tag="onesk")
        nc.vector.memset(onesk[64:65, :], 1.0)

        QT = const.tile([P, NT], DT_ATTN, tag="QT")
        KT = const.tile([P, NT], DT_ATTN, tag="KT")
        VT = const.tile([P, NT], DT_ATTN, tag="VT")
        # V natural layout per 128-k chunk, heads split with a ones column
        # each: [V0(0:64) | 1(64) | V1(65:129) | 1(129)]
        V = const.tile([P, B * 16, 130], DT_ATTN, tag="V")
        nc.vector.memset(V[:, :, 64:65], 1.0)
        nc.vector.memset(V[:, :, 129:130], 1.0)
        # attn output (transposed, pre-Wo): head0 on p0:64, head1 on p64:128
        outT = const.tile([P, NT], DT_ATTN, tag="outT")

        # ---- stage A: projections ----
        # kc=0's x tile flows through a DVE copy so the group-opening matmul's
        # two deps (fresh x data + psum slot recycle) merge into one DVE wait;
        # kc>0 matmuls wait only on their own x DMA lane.
        def proj(xt_d, wbase, nts, dest, granular=False, n0=None, w=None):
            _lab(f"proj_{'qkv'[[WQ_BASE, WK_BASE, WV_BASE].index(wbase)] if wbase != WO_BASE else 'o'}")
            # nts: 1 or 2 consecutive 512-wide tiles; x arrives as two
            # 4-chunk batched DMAs (HWDGE issue cost ~625ns/op dominates
            # many-small-DMA schedules; descriptor count is unchanged).
            # n0/w override the token range (trim K/V past valid_len).
            xt_r = xt_d.rearrange("(kc p) n -> p kc n", p=P)
            if w is None:
                w = 512 * len(nts)
            if n0 is None:
                n0 = nts[0] * 512
            pss = [ps_pw.tile([P, 512], dt.float32, tag="pw", name="ps_proj")
                   for _ in nts]
            HK = DK // 2
            xts = []
            for g in range(2):
                xt = xpool.tile([P, HK, w], DT_IN, tag=f"xt{g}", name="xt")
                if granular and g == 0:
                    # two half-group DMAs for the very first call only: gets
                    # PE started earlier; 256KB transfers still hide the
                    # 625ns HWDGE issue overhead (128KB ones would not)
                    for kc in range(0, HK, 2):
                        nc.sync.dma_start(xt[:, kc:kc + 2, :],
                                          xt_r[:, g * HK + kc:g * HK + kc + 2,
                                               n0:n0 + w])
                else:
                    nc.sync.dma_start(xt, xt_r[:, g * HK:(g + 1) * HK,
                                               n0:n0 + w])
                xts.append(xt)
            for kc in range(DK):
                xt = xts[kc // HK][:, kc % HK, :]
                for i in range(len(nts)):
                    wi = min(512, w - i * 512)
                    nc.tensor.matmul(
                        pss[i][:, 0:wi],
                        lhsT=wall[:, wbase + kc * F:wbase + (kc + 1) * F],
                        rhs=xt[:, i * 512:i * 512 + wi],
                        start=(kc == 0), stop=(kc == DK - 1))
            for i, nt in enumerate(nts):
                wi = min(512, w - i * 512)
                nc.vector.tensor_copy(out=dest[:, n0 + i * 512:n0 + i * 512 + wi],
                                      in_=pss[i][:, 0:wi])

        def pairs(lst):
            return [lst[i:i + 2] for i in range(0, len(lst), 2)]

        def v_nat(b, kcl):
            _lab(f"vT_{b}")
            # V natural (k on partitions) via PE transpose of a VT chunk
            g = b * 16 + kcl
            pst = ps_score.tile([P, P], DT_ATTN, tag="sc", name="pst")
            nc.tensor.transpose(pst, VT[:, g * 128:(g + 1) * 128], ident)
            nc.vector.tensor_copy(out=V[:, g, 0:64], in_=pst[:, 0:64])
            nc.vector.tensor_copy(out=V[:, g, 65:129], in_=pst[:, 64:128])

        def scores_exp(b, qt, kcl):
            # scores + exp for one k-chunk; returns the at tile. Emitted
            # ahead of the previous qt's normalize chain, this is PE/ACT
            # work with no dependence on the av accumulators.
            _lab(f"sc_{b}")
            q0 = b * S + qt * 512
            g = b * 16 + kcl
            k0 = b * S + kcl * 128
            sc2 = ps_score.tile([P, 2, 512], dt.float32, tag="sc", name="sc2")
            nc.tensor.matmul(sc2[:, 0, :], lhsT=KT[0:64, k0:k0 + 128],
                             rhs=QT[0:64, q0:q0 + 512])
            nc.tensor.matmul(sc2[:, 1, :], lhsT=KT[64:128, k0:k0 + 128],
                             rhs=QT[64:128, q0:q0 + 512])
            at = apool.tile([P, 2, 512], DT_ATTN, tag="at", name="at")
            bias = mask_sb[:, g:g + 1]
            nc.scalar.activation(at.rearrange("p a n -> p (a n)"),
                                 sc2.rearrange("p a n -> p (a n)"),
                                 mybir.ActivationFunctionType.Exp,
                                 bias=bias, scale=0.125)
            return at

        def av_mms(b, av0, av1, kcl, at):
            _lab(f"av_{b}")
            g = b * 16 + kcl
            sp = (kcl == KC[b] - 1)
            nc.tensor.matmul(av0[0:65], lhsT=V[:, g, 0:65],
                             rhs=at[:, 0, :], start=(kcl == 0), stop=sp)
            nc.tensor.matmul(av1[0:65], lhsT=V[:, g, 65:130],
                             rhs=at[:, 1, :], start=(kcl == 0), stop=sp)

        def attn_chunks(b, qt, av0, av1, kcls, pre=()):
            for i, kcl in enumerate(kcls):
                at = pre[i] if i < len(pre) else scores_exp(b, qt, kcl)
                av_mms(b, av0, av1, kcl, at)

        def avc_copies(av0, av1):
            # drain the av accumulators to SBUF right after the last AV
            # matmul: frees the 2 av PSUM banks for the next q tile and
            # lets the whole normalize chain run deferred (SBUF x PSUM
            # TensorTensor is legal; PSUM x PSUM is not)
            avc0 = rpool.tile([65, 512], DT_ATTN, tag="avc0", name="avc0")
            avc1 = rpool.tile([65, 512], DT_ATTN, tag="avc1", name="avc1")
            nc.scalar.copy(out=avc0, in_=av0[0:65])
            nc.scalar.copy(out=avc1, in_=av1[0:65])
            return avc0, avc1

        def norm_qt(b, qt, avc0, avc1, av0=None, av1=None):
            _lab(f"nrm_{b}")
            q0 = b * S + qt * 512
            # normalize both heads: rowsums at avc0[64] / avc1[64]. 1/r on
            # DVE (fp16, values <= 1), K=1 fp16 matmuls broadcast each 1/r
            # over 64 partitions; DVE multiplies SBUF x PSUM. head1's
            # normalized tile is shifted to partitions 64:128 with a K=64
            # identity matmul (PE is the only partition mover) so Wo
            # contracts both heads in single K=128 matmuls.
            rinv = rpool.tile([P, 2, 512], DT_ATTN, tag="rinv", name="rinv")
            r0src = avc0[64:65, :] if av0 is None else av0[64:65, :]
            r1src = avc1[64:65, :] if av1 is None else av1[64:65, :]
            with nc.allow_low_precision(reason="1/rowsum <= 1 fits fp16"):
                nc.vector.reciprocal(out=rinv[64:65, 0, :], in_=r0src)
                nc.vector.reciprocal(out=rinv[64:65, 1, :], in_=r1src)
            bc0 = ps_pw.tile([P, 512], dt.float32, tag="pw", name="bc0")
            nc.tensor.matmul(bc0[0:64], lhsT=onesk[64:65, :],
                             rhs=rinv[64:65, 0, :])
            nc.vector.tensor_mul(out=outT[0:64, q0:q0 + 512],
                                 in0=avc0[0:64], in1=bc0[0:64])
            bc1 = ps_pw.tile([P, 512], dt.float32, tag="pw", name="bc1")
            nc.tensor.matmul(bc1[0:64], lhsT=onesk[64:65, :],
                             rhs=rinv[64:65, 1, :])
            tmp1 = rpool.tile([DH, 512], DT_ATTN, tag="tmp1", name="tmp1")
            nc.vector.tensor_mul(out=tmp1, in0=avc1[0:64], in1=bc1[0:64])
            mv = ps_pw.tile([P, 512], dt.float32, tag="pw", name="mv")
            nc.tensor.matmul(mv[64:128], lhsT=ident[0:64, 0:64], rhs=tmp1)
            nc.vector.tensor_copy(out=outT[64:128, q0:q0 + 512],
                                  in_=mv[64:128])

        def wo_units(b, qt, last=False):
            # Wo for one q tile as 4 deferred closures (one 128-row s-chunk
            # each: 2 K=128 matmuls + DVE drains + out DMA every 2 chunks).
            # Interleaved into the NEXT q tile's attention so the PSUM
            # drains overlap exp-paced slack instead of stalling PE.
            q0 = b * S + qt * 512
            gs0 = q0 // 128
            ost = opool.tile([P, 4, D], DT_OUT, tag="ost", name="ost")

            def unit(sci):
                def emit():
                    _lab(f"wo_{b}")
                    r0 = (gs0 + sci) * 128
                    for half in range(2):
                        pw = ps_pw.tile([P, 512], dt.float32, tag="pw",
                                        name="pw")
                        w_sl = slice(half * 512, (half + 1) * 512)
                        nc.tensor.matmul(pw, lhsT=outT[:, r0:r0 + 128],
                                         rhs=wall[:, WO_BASE + w_sl.start:
                                                  WO_BASE + w_sl.stop])
                        if last and half == 1:
                            nc.scalar.copy(out=ost[:, sci, w_sl], in_=pw)
                        else:
                            nc.vector.tensor_copy(out=ost[:, sci, w_sl],
                                                  in_=pw)
                    if last:
                        nc.sync.dma_start(
                            out_d.rearrange(
                                "(g p) n -> p g n",
                                p=P)[:, gs0 + sci:gs0 + sci + 1, :],
                            ost[:, sci:sci + 1, :])
                    elif sci == 1 or sci == 3:
                        nc.sync.dma_start(
                            out_d.rearrange(
                                "(g p) n -> p g n",
                                p=P)[:, gs0 + sci - 1:gs0 + sci + 1, :],
                            ost[:, sci - 1:sci + 1, :])
                return emit
            return [unit(s) for s in range(4)]

        # ---- per batch: segmented K/V projection with qt0's attention
        # chunks interleaved, then qt1..3 streaming; Wo work of each qt is
        # deferred into the next qt's attention loop ----
        wo_todo = []

        def attn_seq(b, qt, av0, av1, kcls, i0=0, pre=()):
            for i, kcl in enumerate(kcls):
                at = pre[i] if i < len(pre) else scores_exp(b, qt, kcl)
                av_mms(b, av0, av1, kcl, at)
                if i0 + i >= 2 and wo_todo:
                    wo_todo.pop(0)()

        pending = None  # (b, qt, av0, av1) whose normalize awaits cover

        for b in range(B):
            n_kv_tiles = math.ceil(KC[b] * 128 / 512)
            av0 = ps_av.tile([P, 512], dt.float32, tag="av", name="av0")
            av1 = ps_av.tile([P, 512], dt.float32, tag="av", name="av1")
            for si in range(n_kv_tiles):
                t = b * 4 + si
                g = (b == 0 and si == 0)
                n0 = b * S + si * 512
                wkv = min(512, KC[b] * 128 - si * 512)
                c_lo = si * 4
                c_hi = min(c_lo + 4, KC[b])
                pre0 = ()
                w_sec(WK_BASE)
                if not (b == 1 and si == 0):
                    proj(xtk_d, WK_BASE, [t], KT, granular=g, n0=n0, w=wkv)
                if si == 0:
                    # Q + scores before V: scores need only K and Q, so PE
                    # streams them while V's x tiles are still in flight
                    w_sec(WQ_BASE)
                    proj(xtq_d, WQ_BASE, [b * 4], QT)
                    if b == 0:
                        nc.sync.dma_start(mask_raw, mask_d[:, :])
                        nc.scalar.copy(out=mask_sb, in_=mask_raw)
                    if pending is not None:
                        # b's first segment covers the previous batch's
                        # last normalize/Wo chain
                        pn = pending
                        wo_todo.append(lambda pn=pn: norm_qt(*pn))
                        wo_todo.extend(wo_units(pn[0], pn[1]))
                        pending = None
                    pre0 = [scores_exp(b, 0, kcl)
                            for kcl in range(c_lo, c_hi)]
                w_sec(WV_BASE)
                proj(xtv_d, WV_BASE, [t], VT, n0=n0, w=wkv)
                for kcl in range(c_lo, c_hi):
                    v_nat(b, kcl)
                attn_seq(b, 0, av0, av1, list(range(c_lo, c_hi)), i0=c_lo,
                         pre=pre0)
                if si == 1:
                    w_sec(WO_BASE, D)
                    proj(xtq_d, WQ_BASE, [b * 4 + 1], QT)
            for qt in range(3):
                if qt < 2:
                    # Q projection for qt+2: doubles as cover for this
                    # boundary's normalize chain, and its PSUM drain is
                    # long done before that tile's first scores
                    proj(xtq_d, WQ_BASE, [b * 4 + qt + 2], QT)
                elif b == 0:
                    # next batch's first K tile: covers this boundary and
                    # prefetches its x during b0's last attention tile
                    wkv1 = min(512, KC[1] * 128)
                    proj(xtk_d, WK_BASE, [4], KT, n0=S, w=wkv1)
                c0, c1 = avc_copies(av0, av1)
                wo_todo.append(lambda c0=c0, c1=c1, b=b, qt=qt:
                               norm_qt(b, qt, c0, c1))
                wo_todo.extend(wo_units(b, qt))
                av0 = ps_av.tile([P, 512], dt.float32, tag="av", name="av0")
                av1 = ps_av.tile([P, 512], dt.float32, tag="av", name="av1")
                attn_seq(b, qt + 1, av0, av1, list(range(KC[b])))
            for u in wo_todo:
                u()
            wo_todo = []
            c0, c1 = avc_copies(av0, av1)
            pending = (b, 3, c0, c1)
        if pending is not None:
            norm_qt(*pending)
            for u in wo_units(pending[0], pending[1], last=True):
                u()

    _legalize_waits(nc)
    return nc


def _legalize_waits(nc):
    """This walrus build accepts at most ONE sync-wait command per
    instruction, while Tile emits up to a dozen (e.g. the kernel-tail
    drain). Legalize by splitting: excess waits are hoisted onto
    same-engine Drain instructions inserted immediately before the
    offender — same-engine program order makes this semantically
    identical. Patched module is served via nc.to_json_bytes."""
    import json as _json

    raw = nc.to_json_bytes()
    d = _json.loads(raw)
    template = None
    for fn in d.get("functions", []):
        for blk in fn.get("blocks", []):
            for inst in blk.get("instructions", []):
                if inst.get("opcode") == "Drain":
                    template = inst
                    break
            if template:
                break
        if template:
            break
    assert template is not None, "no Drain template found"

    counter = [0]

    def carrier(engine, wait):
        counter[0] += 1
        c = _json.loads(_json.dumps(template))
        c["name"] = f"I-waitfix-{counter[0]}"
        c["engine"] = engine
        c["sync_info"] = {"on_update": [], "on_wait": [wait]}
        c["ins"] = []
        c["outs"] = []
        return c

    nfix = 0
    for fn in d.get("functions", []):
        for blk in fn.get("blocks", []):
            out = []
            for inst in blk.get("instructions", []):
                si = inst.get("sync_info")
                waits = (si or {}).get("on_wait") or []
                if len(waits) > 1:
                    for w in waits[:-1]:
                        out.append(carrier(inst["engine"], w))
                    si["on_wait"] = [waits[-1]]
                    nfix += 1
                out.append(inst)
            blk["instructions"] = out

    patched = _json.dumps(d).encode()
    nc.to_json_bytes = lambda: patched


def _prep_host(queries, keys, values, Wq, Wk, Wv, Wo, valid_lens, cfg):
    np_in = {"bfloat16": ml_dtypes.bfloat16, "float32": np.float32,
             "float32r": np.float32, "float16": np.float16}[cfg["dt_in"]]
    L = [int(valid_lens[0]), int(valid_lens[1])]
    KC = tuple(min(16, (l + 127) // 128) for l in L)

    def t2(x):  # (B,S,D) -> (D, B*S)
        return np.ascontiguousarray(
            np.asarray(x, np.float32).reshape(NT, D).T).astype(np_in)

    xtq, xtk, xtv = t2(queries), t2(keys), t2(values)
    maskt = np.full((P, B * 16), NEG, np.float32)
    for b in range(B):
        for c in range(16):
            ks = c * 128 + np.arange(P)
            maskt[:, b * 16 + c] = np.where(ks < L[b], 0.0, NEG)

    Wq = np.asarray(Wq, np.float32)
    Wk = np.asarray(Wk, np.float32)
    Wv = np.asarray(Wv, np.float32)
    Wo = np.asarray(Wo, np.float32)

    def packw(Wx, cs):  # [D, F] slice -> [P, DK*F], p-major rows (2KB elems)
        return Wx[:, cs].reshape(DK, P, F).transpose(1, 0, 2).reshape(P, DK * F)

    in_maps = []
    for c in range(N_CORES):
        cs = slice(c * F, (c + 1) * F)
        wall = np.concatenate(
            [packw(Wq, cs), packw(Wk, cs), packw(Wv, cs), Wo[cs, :]],
            axis=1)
        in_maps.append({
            "xtq": xtq, "xtk": xtk, "xtv": xtv,
            "wall": np.ascontiguousarray(wall).astype(np_in),
            "maskt": maskt,
        })
    return KC, in_maps


DEFAULT_CFG = {"dt_in": "float16", "dt_attn": "float16", "dt_out": "float16"}

LAST_RESULTS = None


def kernel(queries, keys, values, Wq, Wk, Wv, Wo, valid_lens):
    global LAST_RESULTS
    from concourse.bass_utils import run_bass_kernel_spmd

    cfg = dict(DEFAULT_CFG)
    if os.environ.get("MHA_CFG"):
        for kv in os.environ["MHA_CFG"].split(","):
            k, v = kv.split("=")
            cfg[k] = v

    KC, in_maps = _prep_host(queries, keys, values, Wq, Wk, Wv, Wo,
                             valid_lens, cfg)
    key = (KC, tuple(sorted(cfg.items())))
    if key not in _CACHE:
        _CACHE[key] = _build_program(KC, cfg)
    nc = _CACHE[key]

    trace = bool(os.environ.get("MHA_TRACE"))
    res = run_bass_kernel_spmd(nc, in_maps, core_ids=list(range(N_CORES)),
                               trace=trace)
    LAST_RESULTS = res
    acc = np.zeros((NT, D), np.float32)
    for r in res.results:
        acc += np.asarray(r["out_part"], np.float32)
    return acc.reshape(B, S, D)
